# revision 13
# baseline (speedup 1.0000x reference)
"""Additive (Bahdanau) attention kernel for 8 Trainium2 NeuronCores.

Problem (hardcoded shapes):
  key   [4, 512, 256] f32    que   [4, 512, 256] f32   value [4, 512, 256] f32
  W_k/W_q [256, 128] f32     b_k/b_q [128] f32         w_v [128] f32, b_v scalar
  valid_lens [4, 512] int32
  out[b,k,:] = softmax_t(mask(w_v . tanh(kf[b,k,:] + qf[b,t,:]))) @ value[b]

Separable approximation (same spirit as v1, one rank cheaper on the ACT
engine):

  tanh(x+y) ~ c0(x) + cL(x)*y + sum_m c_m(x) * tanh(y + beta_m),  m = 1..4

(c0 is free because softmax is shift-invariant per row; the LINEAR basis
function y is free on-device because qfT is already in SBUF).  Then

  scores[k,t] ~ sum_{(m,h)} [w_v[h] c_m(kf[k,h])] * basis_m(qf[t,h])
             = (G @ H^T)[k,t],   5 accumulating 128-deep matmuls

G is evaluated on the host (same spirit as the host-side projections);
H needs only FOUR on-device ACT passes  HT[m] = Tanh(qfT + beta_m).

v2 layout: scores are computed TRANSPOSED, per 128-query-position chunk:

  ps_sc[c][t, k] = sum_h HT[m][h, 128c+t] * GT[m][h, k]

so  attnT = exp(ps_sc) * mask  lands directly in the orientation the
output matmul wants as its stationary operand (no PE transposes at all):

  ps_o[khalf] += attnT[c][:, khalf]^T @ value_chunk[c]   (ones column
                                                          gives rowsum)
  out = ps_o[:, :256] * recip(rowsum)

Sharding: core c owns batch b = c//2 and half the TK rows, dealt from a
per-batch DESCENDING sort of valid_lens.  That sort makes per-chunk
validity a PREFIX over k: chunk c only needs columns k with
valid_lens[k] > 128c, so its width is trimmed to width(c) (~256/208/144/
72 instead of 4x256).  Masking shrinks to a narrow "band" of columns
whose valid_lens falls inside the chunk - a single small in-place DVE
multiply per chunk; fully-valid columns skip masking entirely.

DMA uses THREE queues (two HWDGE rings + the gpsimd software DGE):
  ACT ring: qfT cols 0:256, GT rounds 0-1, output half 0
  SP  ring: qfT cols 256:512, GT round 2, GT rounds 3-4, output half 1
  swdge:    band masks, value chunks (+ones columns)
A dummy 8-element Exp leads the ACT queue so the ~1.3us ACT_TABLE_LOAD
(one table set covers Tanh and Exp) overlaps the DMAs.
"""

from contextlib import ExitStack

import numpy as np
import ml_dtypes

import concourse.bass as bass
import concourse.bacc as bacc
import concourse.tile as tile
from concourse import mybir
from concourse.bass_utils import run_bass_kernel_spmd
from concourse.instruction_name_ordered_set import InstructionNameOrderedSet

F32 = mybir.dt.float32
BF16 = mybir.dt.bfloat16
NPBF16 = ml_dtypes.bfloat16

B, TK, TQ = 4, 512, 512
KEYSIZE, QUESIZE, VALSIZE, H = 256, 256, 256, 128
NCORES = 8
R = (B * TK) // NCORES          # 256 rows per core
NTANH = 4                       # shifted-tanh basis functions (device ACT)
NM = NTANH + 1                  # + the linear basis (qfT itself)
NC4 = TQ // 128                 # query-position chunks of 128
BETAS = (-1.40484853, -0.44880348, 0.46442655, 1.42564936)
GRID_N = 801                    # fit grid resolution
GRID_X = 9.0                    # grid covers [-X, X]; |kf|,|qf| < 5 in practice
SIGMA = 1.0322711               # Gaussian weight width of the LSQ fit
VP = VALSIZE + 4                # value chunk width incl. ones column + pad

_basis_cache = None
_program_cache: dict[tuple, bacc.Bacc] = {}


def _basis():
    """Weighted LSQ fit tanh(x+y) ~ c0(x) + cL(x) y + sum_m c_m(x) tanh(y+b_m)
    on a grid with Gaussian weights (kf/qf entries are ~N(0,1)).  c0 is
    discarded: it only shifts each softmax row by a constant.  Returns the
    grid and the coefficient table cm [GRID_N, NM] with the LINEAR basis
    coefficient cL in column 0."""
    global _basis_cache
    if _basis_cache is None:
        xs = np.linspace(-GRID_X, GRID_X, GRID_N)
        w = np.exp(-0.5 * (xs / SIGMA) ** 2)
        w += 1e-7 * w.max()
        Phi = np.concatenate(
            [np.ones((GRID_N, 1)), xs[:, None],
             np.tanh(xs[:, None] + np.array(BETAS)[None, :])],
            axis=1)
        sw = np.sqrt(w)[:, None]
        F = np.tanh(xs[:, None] + xs[None, :])
        C, *_ = np.linalg.lstsq(Phi * sw, F.T * sw, rcond=None)
        cm = C.T[:, 1:]                      # [GRID_N, NM]: [lin, tanh x4]
        _basis_cache = (xs, np.ascontiguousarray(cm))
    return _basis_cache


def _build_program(widths: tuple, nfulls: tuple) -> bacc.Bacc:
    nc = bacc.Bacc()

    bands = tuple(w - n for w, n in zip(widths, nfulls))
    boffs = tuple(int(np.sum(bands[:c])) for c in range(NC4 + 1))
    SBW = boffs[NC4]

    qfT_h = nc.declare_dram_parameter("qfT", [H, TQ], BF16, isOutput=False)
    GT_h = nc.declare_dram_parameter("GT", [H, NM * R], BF16, isOutput=False)
    vp_h = nc.declare_dram_parameter("value_plus", [128, NC4 * VP], BF16,
                                     isOutput=False)
    mb_h = nc.declare_dram_parameter("maskband", [128, max(SBW, 8)], BF16,
                                     isOutput=False)
    out_h = nc.declare_dram_parameter("out", [R, VALSIZE], BF16, isOutput=True)

    out_v = out_h[:].rearrange("(s p) v -> s p v", p=128)       # [2,128,V]
    GT_v = GT_h[:].rearrange("h (m r) -> h m r", m=NM)

    # which chunks feed each k-half of the output accumulation
    half_cs = [[c for c in range(NC4) if widths[c] > 128 * hf] for hf in (0, 1)]

    with ExitStack() as ctx:
        tc = ctx.enter_context(tile.TileContext(nc))
        consts = ctx.enter_context(tc.tile_pool(name="consts", bufs=1))
        smax = ctx.enter_context(tc.tile_pool(name="smax", bufs=2))
        psum_sc = ctx.enter_context(tc.tile_pool(name="psum_sc", bufs=1, space="PSUM"))
        psum_out = ctx.enter_context(tc.tile_pool(name="psum_out", bufs=2, space="PSUM"))

        sb_qfT = consts.tile([128, TQ], BF16, name="qft")
        sb_GT = consts.tile([128, NM, R], BF16, name="gt")
        sb_HT = [consts.tile([128, TQ], BF16, name=f"ht{m}") for m in range(NTANH)]
        sb_vp = consts.tile([128, NC4, VP], BF16, name="vp")
        sb_mb = consts.tile([128, max(SBW, 8)], BF16, name="mb")
        sb_warm = consts.tile([1, 8], F32)
        sb_beta = consts.tile([128, NTANH], F32, name="beta")

        # act-table warm-up first so the ~1.3us table load overlaps the DMAs
        nc.vector.memset(sb_warm, 0.0)
        for m in range(NTANH):
            nc.vector.memset(sb_beta[:, m:m + 1], float(BETAS[m]))
        nc.scalar.activation(
            out=sb_warm, in_=sb_warm, func=mybir.ActivationFunctionType.Exp)

        # DMA: three queues, chunks ordered by first use.  The first matmul
        # round (the linear basis) needs GT rounds 0-1 and qfT - one on each
        # HW ring, first in line; everything else queues behind them.
        nc.scalar.dma_start(out=sb_GT[:, 0:2, :], in_=GT_v[:, 0:2, :])
        nc.sync.dma_start(out=sb_qfT, in_=qfT_h[:])
        nc.scalar.dma_start(out=sb_GT[:, 2:3, :], in_=GT_v[:, 2:3, :])
        nc.sync.dma_start(out=sb_GT[:, 3:NM, :], in_=GT_v[:, 3:NM, :])
        nc.gpsimd.dma_start(out=sb_mb, in_=mb_h[:])
        nc.gpsimd.dma_start(
            out=sb_vp, in_=vp_h[:].rearrange("p (c v) -> p c v", c=NC4))

        # HT[m] = tanh(qfT + beta_m) on device
        for m in range(NTANH):
            nc.scalar.activation(
                out=sb_HT[m], in_=sb_qfT,
                func=mybir.ActivationFunctionType.Tanh, bias=sb_beta[:, m:m + 1])

        # transposed scores, m-major so matmul rounds overlap the TANH chain.
        # Chunks are PAIRED into shared psum-bank tiles (c0+c1, c2+c3):
        # consecutive matmuls then write the same bank, which runs the PE at
        # full rate (bank-switching between matmuls costs ~2x).
        woffs = [0] + list(np.cumsum(widths))
        pair_of = []            # chunk -> (pair tile idx, col offset)
        ps_pair = []
        for pr, cs in enumerate(((0, 1), (2, 3))):
            pw = sum(widths[c] for c in cs)
            ps_pair.append(
                psum_sc.tile([128, max(pw, 8)], F32, tag=f"sc{pr}", name=f"ps_sc{pr}"))
            off = 0
            for c in cs:
                pair_of.append((pr, off))
                off += widths[c]
        # start=True zeroes the WHOLE 2KB psum bank, so only the first chunk
        # of each pair carries it; the partner's first matmul accumulates
        # onto the just-zeroed region (forced to run after via a nosync dep).
        basis = [sb_qfT] + sb_HT
        bank_zero_mm = {}
        for m in range(NM):
            for c in range(NC4):
                if widths[c] == 0:
                    continue
                pr, off = pair_of[c]
                first_of_pair = pr not in bank_zero_mm
                inst = nc.tensor.matmul(
                    ps_pair[pr][:, off:off + widths[c]],
                    basis[m][:, c * 128:(c + 1) * 128],
                    sb_GT[:, m, 0:widths[c]],
                    start=(m == 0 and first_of_pair),
                    stop=(m == NM - 1),
                    skip_group_check=True,
                )
                if m == 0:
                    if first_of_pair:
                        bank_zero_mm[pr] = inst
                    else:
                        deps = InstructionNameOrderedSet()
                        deps.add(bank_zero_mm[pr].ins.name)
                        inst.ins.add_nosync_dependencies_from(deps)

        # |scores| <= ~12 so Exp never overflows f32/bf16: no max-shift.
        # One Exp per pair tile straight out of PSUM; only the band columns
        # (valid_lens inside the chunk) need masking - one small in-place
        # DVE multiply per chunk.
        e_pair = []
        for pr, cs in enumerate(((0, 1), (2, 3))):
            pw = sum(widths[c] for c in cs)
            e = smax.tile([128, max(pw, 8)], BF16, tag=f"e{pr}", name=f"e{pr}")
            nc.scalar.activation(
                out=e[:, 0:pw], in_=ps_pair[pr][:, 0:pw],
                func=mybir.ActivationFunctionType.Exp)
            for c in cs:
                _, off = pair_of[c]
                if bands[c] > 0:
                    nc.vector.tensor_mul(
                        e[:, off + nfulls[c]:off + widths[c]],
                        e[:, off + nfulls[c]:off + widths[c]],
                        sb_mb[:, boffs[c]:boffs[c + 1]])
            e_pair.append(e)

        # output accumulation: ps_o[half] += attnT[c][:,half]^T @ value[c].
        # h-major so consecutive matmuls chain into the SAME psum bank -
        # bank-switching between matmuls costs ~2x on the PE.
        ps_o = {}
        for hf in (0, 1):
            ps_o[hf] = psum_out.tile([128, VP], F32, tag=f"o{hf}", name=f"ps_o{hf}")
        for hf in (0, 1):
            for c in half_cs[hf]:
                lo = hf * 128
                hi = min(widths[c], lo + 128)
                pr, off = pair_of[c]
                nc.tensor.matmul(
                    ps_o[hf][0:hi - lo, :],
                    e_pair[pr][:, off + lo:off + hi], sb_vp[:, c, :],
                    start=(c == half_cs[hf][0]), stop=(c == half_cs[hf][-1]),
                    skip_group_check=True,
                )

        for hf in (0, 1):
            # ones-column of value_plus makes ps_o[:, VALSIZE] the rowsum
            rinv = smax.tile([128, 1], F32, tag=f"rinv{hf}", name=f"rinv{hf}")
            nc.vector.reciprocal(out=rinv, in_=ps_o[hf][:, VALSIZE:VALSIZE + 1])
            sb_o = smax.tile([128, VALSIZE], BF16, tag=f"sb_o{hf}", name=f"sb_o{hf}")
            if hf == 0:
                # half 0 finishes first: scale on ACT, store on the ACT ring
                nc.scalar.activation(
                    out=sb_o, in_=ps_o[hf][:, 0:VALSIZE],
                    func=mybir.ActivationFunctionType.Copy, scale=rinv[:, 0:1])
                nc.scalar.dma_start(out=out_v[hf], in_=sb_o)
            else:
                nc.vector.tensor_scalar_mul(
                    out=sb_o, in0=ps_o[hf][:, 0:VALSIZE], scalar1=rinv[:, 0:1])
                nc.sync.dma_start(out=out_v[hf], in_=sb_o)

    nc.compile()
    return nc


def _prepare(key, que, value, W_k, b_k, W_q, b_q, w_v, b_v, valid_lens):
    """Host prep: projections, sort/deal rows, basis evaluation, in_maps."""
    xs, cm = _basis()
    kf = key @ W_k + b_k                    # [B,TK,H] f32
    qf = que @ W_q + b_q                    # [B,TQ,H] f32

    rows_of_core = []
    vls = []
    for b in range(B):
        order = np.argsort(-valid_lens[b], kind="stable")
        for h in range(2):
            rows = order[h::2]
            rows_of_core.append(rows)
            vls.append(valid_lens[b][rows])

    # common (max-over-cores) prefix widths per 128-query chunk, and the
    # common fully-valid prefix (min over cores) that can skip masking
    widths = []
    nfulls = []
    for c in range(NC4):
        w = max(int((vl > 128 * c).sum()) for vl in vls)
        n = min(int((vl >= 128 * (c + 1)).sum()) for vl in vls)
        w = min(-(-w // 8) * 8, R)
        n = min((n // 8) * 8, w)
        widths.append(w)
        nfulls.append(n)
    widths = tuple(widths)
    nfulls = tuple(nfulls)
    bands = tuple(w - n for w, n in zip(widths, nfulls))
    SBW = int(np.sum(bands))

    in_maps = []
    qfT_of_batch = {}
    vp_of_batch = {}
    p = np.arange(128)
    for c in range(NCORES):
        b = c // 2
        rows = rows_of_core[c]
        vl = vls[c]
        kfr = kf[b][rows]                   # [R, H]
        GT = np.empty((H, NM, R), NPBF16)
        for m in range(NM):
            GT[:, m, :] = (np.interp(kfr, xs, cm[:, m]) * w_v[None, :]).T
        if b not in qfT_of_batch:
            qfT_of_batch[b] = np.ascontiguousarray(qf[b].T).astype(NPBF16)
            vpb = np.zeros((128, NC4 * VP), NPBF16)
            for c4 in range(NC4):
                vpb[:, c4 * VP:c4 * VP + VALSIZE] = value[b][c4 * 128:(c4 + 1) * 128]
                vpb[:, c4 * VP + VALSIZE] = 1.0
            vp_of_batch[b] = vpb

        # band masks: mb[p, boff+j] = (128c + p) < vl[nfull+j]
        mb = np.zeros((128, max(SBW, 8)), NPBF16)
        off = 0
        for c4 in range(NC4):
            if bands[c4] == 0:
                continue
            vlb = vl[nfulls[c4]:widths[c4]]
            mb[:, off:off + bands[c4]] = (
                (128 * c4 + p)[:, None] < vlb[None, :])
            off += bands[c4]

        in_maps.append({
            "qfT": qfT_of_batch[b],
            "GT": np.ascontiguousarray(GT.reshape(H, NM * R)),
            "value_plus": vp_of_batch[b],
            "maskband": mb,
        })
    return widths, nfulls, in_maps, rows_of_core


def kernel(key, que, value, W_k, b_k, W_q, b_q, w_v, b_v, valid_lens):
    key = np.asarray(key, np.float32)
    que = np.asarray(que, np.float32)
    value = np.asarray(value, np.float32)
    W_k = np.asarray(W_k, np.float32)
    b_k = np.asarray(b_k, np.float32)
    W_q = np.asarray(W_q, np.float32)
    b_q = np.asarray(b_q, np.float32)
    w_v = np.asarray(w_v, np.float32)
    valid_lens = np.asarray(valid_lens)

    widths, nfulls, in_maps, rows_of_core = _prepare(
        key, que, value, W_k, b_k, W_q, b_q, w_v, b_v, valid_lens)

    cache_key = (widths, nfulls)
    if cache_key not in _program_cache:
        _program_cache[cache_key] = _build_program(widths, nfulls)
    nc = _program_cache[cache_key]

    res = run_bass_kernel_spmd(nc, in_maps, list(range(NCORES)))

    out = np.zeros((B, TK, VALSIZE), np.float32)
    for c in range(NCORES):
        b = c // 2
        out[b][rows_of_core[c]] = np.asarray(
            res.results[c]["out"], dtype=np.float32)
    return out
